# revision 16
# baseline (speedup 1.0000x reference)
"""Trainium2 Bass kernel for nn_AttentionSortNet (sparse_attention).

Computes, per (batch*head) slice:
  sq = bucket-mean(q), sk = bucket-mean(k)          # (64, 64) each
  R  = sq @ sk.T * DIM**-0.5                        # (64, 64)
  r  = (log(relu(R)+eps) + gumbel(u)) / T
  8x log-domain Sinkhorn row/col normalization
  out = exp(r)

Strategy: shard the 32 bh slices across 8 cores (4 bh each, no
communication). On-core, q/k stream in with buckets on the partition
axis (32KB contiguous per partition), bucket sums via strided
free-axis tensor_reduce on DVE, bucket-summary matmuls + transposes on
PE, and Sinkhorn runs in multiplicative form (P /= rowsum; P /= colsum)
so the iteration loop needs no ScalarE activations: row sums are DVE
free-axis reduces, column sums are a single PE matmul against a
block-diagonal ones matrix that also broadcasts the result back to all
partitions.

Built on bacc.Bacc (not raw Bass): its compile pass splits multi-sem
sync waits, which this walrus requires (one wait per instruction).
Constants are built before the TileContext behind an all-engine
barrier so tile instructions don't sync against them.
"""

import sys

for _p in ("/opt/trn_rl_repo",):
    if _p not in sys.path:
        sys.path.insert(0, _p)

import numpy as np

N_CORES = 8
BH = 32
B_PER = BH // N_CORES          # 4 bh per core
SEQ = 8192
D = 64
BUCKET_SIZE = 128
BUCKETS = SEQ // BUCKET_SIZE   # 64 buckets per bh
N_CHUNKS = 4                   # free-dim chunks per 128-bucket tile
CHUNK_W = BUCKET_SIZE // N_CHUNKS   # 32 within-bucket positions per chunk
CHUNK_F = CHUNK_W * D          # 2048 f32 per partition per chunk
EPS = 1e-6
TEMP = 0.7
SINKHORN_ITER = 8
# q/k are reduced to bucket *sums*; fold the two 1/128 mean factors and
# the DIM**-0.5 = 1/8 similarity scale into one constant applied at relu.
R_SCALE = 1.0 / (BUCKET_SIZE * BUCKET_SIZE * 8.0)

_NC_CACHE = None


def _build():
    import concourse.bacc as bacc
    import concourse.mybir as mybir
    import concourse.tile as tile
    from concourse.masks import make_identity
    from contextlib import ExitStack

    fp32 = mybir.dt.float32
    AX = mybir.AxisListType
    AF = mybir.ActivationFunctionType

    nc = bacc.Bacc("TRN2", target_bir_lowering=False, debug=False)

    q = nc.dram_tensor("q", [B_PER, SEQ, D], fp32, kind="ExternalInput")
    k = nc.dram_tensor("k", [B_PER, SEQ, D], fp32, kind="ExternalInput")
    gu = nc.dram_tensor("gumbel_u", [B_PER, BUCKETS, BUCKETS], fp32,
                        kind="ExternalInput")
    out = nc.dram_tensor("out", [B_PER, BUCKETS, BUCKETS], fp32,
                         kind="ExternalOutput")

    # (b, s, d) -> (global bucket row, chunk, chunk payload)
    qv = q.ap().rearrange("b (bk c w) d -> (b bk) c (w d)", bk=BUCKETS, c=N_CHUNKS)
    kv = k.ap().rearrange("b (bk c w) d -> (b bk) c (w d)", bk=BUCKETS, c=N_CHUNKS)
    # bh = 2g + h laid out as partition p = 64h + i, free = (g, j)
    guv = gu.ap().rearrange("(g h) i j -> (h i) g j", h=2)
    outv = out.ap().rearrange("(g h) i j -> (h i) g j", h=2)

    # Constants live outside the TileContext behind a barrier, like the
    # built-in const APs, so tile instructions never sync against them.
    ident_t = nc.alloc_sbuf_tensor("c_ident", [128, 128], fp32)
    ident = ident_t.ap()
    make_identity(nc, ident)
    # block-diagonal ones: colsum matmul lhsT; out[m,f] = sum over the
    # 64-partition block containing m -> column sums pre-broadcast.
    blockwide_t = nc.alloc_sbuf_tensor("c_blockwide", [128, 128], fp32)
    blockwide = blockwide_t.ap()
    nc.gpsimd.memset(blockwide, 0.0)
    nc.gpsimd.memset(blockwide[0:64, 0:64], 1.0)
    nc.gpsimd.memset(blockwide[64:128, 64:128], 1.0)
    epsb_t = nc.alloc_sbuf_tensor("c_eps", [128, 1], fp32)
    nc.gpsimd.memset(epsb_t.ap(), EPS)
    nc.const_aps.aps[(fp32, EPS)] = epsb_t.ap()
    nc.all_engine_barrier()

    with tile.TileContext(nc) as tc, ExitStack() as ctx:
        chunks = ctx.enter_context(tc.tile_pool(name="chunks", bufs=16))
        parts = ctx.enter_context(tc.tile_pool(name="parts", bufs=2))
        sums = ctx.enter_context(tc.tile_pool(name="sums", bufs=4))
        sbt = ctx.enter_context(tc.tile_pool(name="sbt", bufs=4))
        work = ctx.enter_context(tc.tile_pool(name="work", bufs=1))
        small = ctx.enter_context(tc.tile_pool(name="small", bufs=2))
        tpsum = ctx.enter_context(tc.tile_pool(name="tpsum", bufs=2, space="PSUM"))
        rpsum = ctx.enter_context(tc.tile_pool(name="rpsum", bufs=2, space="PSUM"))
        spsum = ctx.enter_context(tc.tile_pool(name="spsum", bufs=2, space="PSUM"))

        def bucket_sums_T(view, tag, t):
            """Stream 128 global bucket rows, return (64=d, 128=rows) SBUF."""
            part = parts.tile([128, N_CHUNKS, D], fp32, tag="part")
            for c in range(N_CHUNKS):
                ch = chunks.tile([128, CHUNK_F], fp32, tag="chunk")
                nc.sync.dma_start(out=ch[:], in_=view[128 * t:128 * (t + 1), c, :])
                nc.vector.reduce_sum(
                    out=part[:, c, :],
                    in_=ch[:].rearrange("p (w d) -> p d w", d=D),
                    axis=AX.X,
                )
            s = sums.tile([128, D], fp32, tag="sums")
            nc.vector.reduce_sum(
                out=s[:], in_=part[:].rearrange("p c d -> p d c"), axis=AX.X
            )
            tp = tpsum.tile([64, 128], fp32, tag="tp")
            nc.tensor.transpose(tp[:], s[:], ident)
            st = sbt.tile([64, 128], fp32, tag=f"T{tag}")
            nc.scalar.copy(st[:], tp[:])
            return st

        # log-domain init tile: p = 64h + i, free = (g, j)
        pln = work.tile([128, 2, BUCKETS], fp32, tag="pln")

        for t in range(2):  # g = t covers bh {2t, 2t+1}
            qT = bucket_sums_T(qv, "q", t)
            kT = bucket_sums_T(kv, "k", t)
            rp = rpsum.tile([128, BUCKETS], fp32, tag="rp")
            for h in range(2):
                nc.tensor.matmul(
                    rp[64 * h:64 * (h + 1), :],
                    qT[:, 64 * h:64 * (h + 1)],
                    kT[:, 64 * h:64 * (h + 1)],
                    start=True, stop=True,
                    tile_position=(0, 64 * h),
                )
            # pln[:, g, :] = relu(R * R_SCALE)
            nc.scalar.activation(
                out=pln[:, t, :], in_=rp[:], func=AF.Relu, scale=R_SCALE
            )

        # gumbel: g = -ln(-ln(u + eps) + eps); r0 = (ln(relu+eps) - u2) / T
        u = work.tile([128, 2, BUCKETS], fp32, tag="u")
        nc.sync.dma_start(out=u[:], in_=guv)
        nc.scalar.activation(out=u[:], in_=u[:], func=AF.Ln, bias=EPS)
        nc.scalar.activation(out=u[:], in_=u[:], func=AF.Ln, bias=EPS, scale=-1.0)
        nc.scalar.activation(out=pln[:], in_=pln[:], func=AF.Ln, bias=EPS)
        nc.vector.tensor_sub(pln[:], pln[:], u[:])
        p = work.tile([128, 2, BUCKETS], fp32, tag="p")
        nc.scalar.activation(out=p[:], in_=pln[:], func=AF.Exp, scale=1.0 / TEMP)

        pf = p[:].rearrange("p g j -> p (g j)")
        for _ in range(SINKHORN_ITER):
            rsum = small.tile([128, 2], fp32, tag="rsum")
            nc.vector.reduce_sum(out=rsum[:], in_=p[:], axis=AX.X)
            rrec = small.tile([128, 2], fp32, tag="rrec")
            nc.vector.reciprocal(rrec[:], rsum[:])
            nc.vector.tensor_mul(
                p[:], p[:], rrec[:].unsqueeze(2).broadcast_to((128, 2, BUCKETS))
            )
            cs = spsum.tile([128, 128], fp32, tag="cs")
            nc.tensor.matmul(cs[:], blockwide, pf, start=True, stop=True)
            crec = small.tile([128, 128], fp32, tag="crec")
            nc.vector.reciprocal(crec[:], cs[:])
            nc.vector.tensor_mul(pf, pf, crec[:])

        nc.gpsimd.dma_start(out=outv, in_=p[:])

    return nc


def _get_nc():
    global _NC_CACHE
    if _NC_CACHE is None:
        _NC_CACHE = _build()
        # Bacc legalization (sync-wait splitting, register allocation)
        # runs in finalize(); the PJRT exec path serializes nc as-is.
        if not _NC_CACHE.is_finalized():
            _NC_CACHE.finalize()
    return _NC_CACHE


def _shard(q, k, gumbel_u):
    return [
        {
            "q": np.ascontiguousarray(q[B_PER * c:B_PER * (c + 1)]),
            "k": np.ascontiguousarray(k[B_PER * c:B_PER * (c + 1)]),
            "gumbel_u": np.ascontiguousarray(gumbel_u[B_PER * c:B_PER * (c + 1)]),
        }
        for c in range(N_CORES)
    ]


def kernel(q, k, gumbel_u, **_unused):
    from concourse.bass_utils import run_bass_kernel_spmd

    q = np.asarray(q, dtype=np.float32)
    k = np.asarray(k, dtype=np.float32)
    gumbel_u = np.asarray(gumbel_u, dtype=np.float32)

    nc = _get_nc()
    res = run_bass_kernel_spmd(nc, _shard(q, k, gumbel_u),
                               core_ids=list(range(N_CORES)))
    return np.concatenate([r["out"] for r in res.results], axis=0)
